# revision 1
# baseline (speedup 1.0000x reference)
"""Trainium2 Bass kernel for a 3-layer edge-typed GNN (message passing + GRU + readout).

Math refactoring (key to the memory-bound regime):
  reference per layer:
    ef = [h[src], h[tgt]]                    # [E, 2H]
    m  = relu(ef @ W1_t) @ W2_t (+biases)    # t = edge type
    messages = segment_sum(m, tgt)
    h = GRU(messages, h)
  Since segment_sum commutes with the second linear layer:
    X_t_src = h @ W1_t[:H] ; X_t_tgt = h @ W1_t[H:] + b1_t    (per-node tables)
    S_t     = segment_sum(relu(X_t_src[src] + X_t_tgt[tgt]), tgt)
    messages = S_sf @ W2_sf + S_fd @ W2_fd (+ counts x b2)
  so ALL per-edge matmuls become per-node matmuls; per edge only
  gather + add + relu + segment-accumulate remain.

Distribution (8 cores):
  - nodes sharded: core k owns nodes [k*6250, (k+1)*6250)
  - edges sharded by TARGET shard -> segment sums are core-local
  - per layer: each core computes X rows for its node shard; the source-side
    halves are AllGathered into a full [50000, 256] table; target-side halves
    stay local. Per-edge src rows are fetched with dma_gather (int16 indices,
    2 row-buckets of 25000 to fit int16), tgt rows from the local table.
  - segment-sum via one-hot matmul: edges sorted by (srcbucket, window) where
    window = 256 contiguous (type,tgt) keys; per 128-edge tile the one-hot
    [128e, 256k] matrix comes from is_equal(iota_row, key); PE accumulates
    S^T[128H, 256k] in PSUM across the window's tiles.
  - GRU + readout data-parallel over node shards, feature-major in SBUF.
"""

import numpy as np

N, E, H, O = 50000, 640000, 128, 2
N_LAYERS = 3
NCORE = 8
NSH = N // NCORE            # 6250 nodes per core
NSHP = 6272                 # padded to 49*128
W = 256                     # segment window width (keys)
KPT = 6400                  # padded keys per type (25 windows of 256)
NWT = KPT // W              # windows per type
NW = 2 * NWT                # 50 windows per core
NBK = 2                     # source-row buckets (int16 index range)
BKS = N // NBK              # 25000 rows per bucket
GCH = 1024                  # idxs per dma_gather call (SWDGE ring holds 1024 descs)
NCH = NSHP // 512           # not exact; node chunking handled explicitly

_CACHE = {}
_RUN_KWARGS = {}
_LAST_RESULT = None


def _preprocess(src, tgt, typ):
    """Build per-core padded edge streams + shared tile structure."""
    core = tgt // NSH
    tgt_l = tgt - core * NSH
    key = typ * KPT + tgt_l
    win = key // W                       # 0..NW-1 (implies type)
    bkt = src // BKS                     # 0..NBK-1

    group = (core * NBK + bkt) * NW + win
    perm = np.argsort(group, kind="stable")
    cnt = np.bincount(group, minlength=NCORE * NBK * NW).reshape(NCORE, NBK, NW)
    T = -(-cnt.max(axis=0) // 128)       # [NBK, NW] tiles per (bucket, window)
    T[0] = np.maximum(T[0], 1)           # guarantee a pass-0 flush per window
    TOT = int(T.sum()) * 128

    starts = np.zeros(NCORE * NBK * NW + 1, np.int64)
    np.cumsum(cnt.reshape(-1), out=starts[1:])
    ss, ts, ks = src[perm], tgt_l[perm], key[perm]

    src_s = np.zeros((NCORE, TOT), np.int16)
    tgt_s = np.zeros((NCORE, TOT), np.int16)
    key_s = np.full((NCORE, TOT), -1.0, np.float32)
    pos = 0
    for bb in range(NBK):
        for ww in range(NW):
            L = int(T[bb, ww]) * 128
            for k in range(NCORE):
                gi = (k * NBK + bb) * NW + ww
                s0, s1 = int(starts[gi]), int(starts[gi + 1])
                n = s1 - s0
                src_s[k, pos:pos + n] = (ss[s0:s1] - bb * BKS).astype(np.int16)
                tgt_s[k, pos:pos + n] = ts[s0:s1].astype(np.int16)
                key_s[k, pos:pos + n] = (ks[s0:s1] - ww * W).astype(np.float32)
            pos += L
    assert pos == TOT

    # gather-call segmentation: contiguous runs per (bucket, type), <= GCH idxs
    calls = []
    pos = 0
    for bb in range(NBK):
        for tt in range(2):
            seg = int(T[bb, tt * NWT:(tt + 1) * NWT].sum()) * 128
            off = 0
            while off < seg:
                ln = min(GCH, seg - off)
                calls.append((pos + off, ln, bb, tt))
                off += ln
            pos += seg

    # wrapped device layouts
    def wrap16(a):   # [TOT] -> [128, TOT//16], 16-row stream replicated x8
        return np.tile(a.reshape(-1, 16).T, (8, 1)).copy()

    sidx = np.stack([wrap16(src_s[k]) for k in range(NCORE)])
    tidx = np.stack([wrap16(tgt_s[k]) for k in range(NCORE)])
    keyv = np.stack([key_s[k].reshape(-1, 128).T.copy() for k in range(NCORE)])

    # per-core per-type in-degree counts (for b2 bias via counts)
    cnt_t = np.zeros((NCORE, 2, NSH), np.float64)
    np.add.at(cnt_t, (core, typ, tgt_l), 1.0)

    return T, TOT, calls, sidx, tidx, keyv, cnt_t


def _build(T, TOT, calls, skip_edge=False, skip_cc=False, edge_mode="full"):
    import concourse.bacc as bacc
    import concourse.mybir as mybir
    import concourse.tile as tile

    f32 = mybir.dt.float32
    bf16 = mybir.dt.bfloat16
    i16 = mybir.dt.int16
    AF = mybir.ActivationFunctionType
    ALU = mybir.AluOpType

    nc = bacc.Bacc("TRN2", target_bir_lowering=False, debug=False,
                   num_devices=NCORE)

    # ---- I/O ----
    hT_d = nc.dram_tensor("hT", [128, NSHP], f32, kind="ExternalInput")
    w1blk_d = nc.dram_tensor("w1blk", [128, 512], f32, kind="ExternalInput")
    w2sf_d = nc.dram_tensor("w2sf", [128, 128], f32, kind="ExternalInput")
    w2fd_d = nc.dram_tensor("w2fd", [128, 128], f32, kind="ExternalInput")
    wihT_d = nc.dram_tensor("wihT", [128, 384], f32, kind="ExternalInput")
    whhT_d = nc.dram_tensor("whhT", [128, 384], f32, kind="ExternalInput")
    gbias_d = nc.dram_tensor("gbias", [128, 3], f32, kind="ExternalInput")
    wr1_d = nc.dram_tensor("wr1", [128, 128], f32, kind="ExternalInput")
    br1_d = nc.dram_tensor("br1", [128, 1], f32, kind="ExternalInput")
    wr2_d = nc.dram_tensor("wr2", [128, 2], f32, kind="ExternalInput")
    br2_d = nc.dram_tensor("br2", [128, 2], f32, kind="ExternalInput")
    b1t_d = nc.dram_tensor("b1t", [128, 256], f32, kind="ExternalInput")
    biasm_d = nc.dram_tensor("biasm", [128, NSHP], f32, kind="ExternalInput")
    iota_d = nc.dram_tensor("iota", [128, W], bf16, kind="ExternalInput")
    sidx_d = nc.dram_tensor("sidx", [128, TOT // 16], i16, kind="ExternalInput")
    tidx_d = nc.dram_tensor("tidx", [128, TOT // 16], i16, kind="ExternalInput")
    keyv_d = nc.dram_tensor("keyv", [128, TOT // 128], f32, kind="ExternalInput")
    out_d = nc.dram_tensor("out", [NSH, 2], f32, kind="ExternalOutput")

    cc_in = nc.dram_tensor("cc_in", [NSH, 256], bf16)
    cc_out = nc.dram_tensor("cc_out", [N, 256], bf16, addr_space="Shared")
    tgt_tab = nc.dram_tensor("tgt_tab", [NSHP, 256], bf16)

    groups = [list(range(NCORE))]

    with tile.TileContext(nc) as tc:
        with tc.tile_pool(name="persist", bufs=1) as pp:
            hT = pp.tile([128, NSHP], f32)
            w1blk = pp.tile([128, 512], f32)
            w2sf = pp.tile([128, 128], f32)
            w2fd = pp.tile([128, 128], f32)
            wihT = pp.tile([128, 384], f32)
            whhT = pp.tile([128, 384], f32)
            gbias = pp.tile([128, 3], f32)
            wr1 = pp.tile([128, 128], f32)
            br1 = pp.tile([128, 1], f32)
            wr2 = pp.tile([128, 2], f32)
            br2 = pp.tile([128, 2], f32)
            b1t = pp.tile([128, 256], f32)
            iota = pp.tile([128, W], bf16)
            sidx = pp.tile([128, TOT // 16], i16)
            tidx = pp.tile([128, TOT // 16], i16)
            keyv = pp.tile([128, TOT // 128], f32)
            ST = pp.tile([128, 2 * KPT], f32)

            for t_, d_ in [(hT, hT_d), (w1blk, w1blk_d), (w2sf, w2sf_d),
                           (w2fd, w2fd_d), (wihT, wihT_d), (whhT, whhT_d),
                           (gbias, gbias_d), (wr1, wr1_d), (br1, br1_d),
                           (wr2, wr2_d), (br2, br2_d), (b1t, b1t_d),
                           (iota, iota_d), (sidx, sidx_d), (tidx, tidx_d),
                           (keyv, keyv_d)]:
                nc.sync.dma_start(t_[:], d_[:])

            for layer in range(N_LAYERS):
                # ---- X phase: per-node tables X = h @ W1blk ----
                with (
                    tc.tile_pool(name=f"xp{layer}", bufs=3) as xp,
                    tc.tile_pool(name=f"xps{layer}", bufs=2, space="PSUM") as xps,
                ):
                    for c in range(NSHP // 128):
                        xpsum = xps.tile([128, 512], f32, tag="xpsum")
                        nc.tensor.matmul(xpsum[:], hT[:, c * 128:(c + 1) * 128],
                                         w1blk[:], start=True, stop=True)
                        xs = xp.tile([128, 256], bf16, tag="xs")
                        nc.vector.tensor_copy(xs[:], xpsum[:, 0:256])
                        xt = xp.tile([128, 256], bf16, tag="xt")
                        nc.vector.tensor_tensor(xt[:], xpsum[:, 256:512], b1t[:],
                                                op=ALU.add)
                        nv = min(128, NSH - c * 128)
                        if nv > 0:
                            nc.sync.dma_start(cc_in[c * 128:c * 128 + nv, :],
                                              xs[:nv, :])
                        nc.sync.dma_start(tgt_tab[c * 128:(c + 1) * 128, :], xt[:])

                if not skip_cc:
                    nc.gpsimd.collective_compute(
                        "AllGather", mybir.AluOpType.bypass,
                        replica_groups=groups,
                        ins=[cc_in[:]], outs=[cc_out[:]],
                    )
                else:
                    with tc.tile_pool(name=f"cp{layer}", bufs=2) as cp:
                        for c in range(NSH // 125):
                            ct_ = cp.tile([125, 256], f32, tag="ct")
                            nc.sync.dma_start(ct_[:], cc_in[c * 125:(c + 1) * 125, :])
                            nc.sync.dma_start(cc_out[c * 125:(c + 1) * 125, :], ct_[:])

                # ---- edge phase ----
                if skip_edge:
                    with tc.tile_pool(name=f"ze{layer}", bufs=1) as zp:
                        zt = zp.tile([128, 2 * KPT], f32, tag="zt")
                        nc.gpsimd.memset(zt[:], 0.0)
                        nc.vector.tensor_copy(ST[:], zt[:])
                with (
                    tc.tile_pool(name=f"eg{layer}", bufs=4) as eg,
                    tc.tile_pool(name=f"et{layer}", bufs=8) as et,
                    tc.tile_pool(name=f"eps{layer}", bufs=4, space="PSUM") as eps,
                ):
                    ci = 0
                    call_off = 0
                    cur_gs = cur_gt = None
                    tile_i = 0
                    if edge_mode == "no_gather":
                        dummy_gs = eg.tile([128, GCH // 128, 128], bf16, tag="gs")
                        dummy_gt = eg.tile([128, GCH // 128, 128], bf16, tag="gt")
                        nc.gpsimd.memset(dummy_gs[:], 0.1)
                        nc.gpsimd.memset(dummy_gt[:], 0.1)
                    for bb in ([] if skip_edge else range(NBK)):
                        for ww in range(NW):
                            Tw = int(T[bb, ww])
                            if Tw == 0:
                                continue
                            st = eps.tile([128, W], f32, tag="st")
                            for t in range(Tw):
                                off = tile_i * 128
                                if (edge_mode != "no_gather"
                                        and ci < len(calls)
                                        and calls[ci][0] == off):
                                    _, ln, cb, ct = calls[ci]
                                    nt = ln // 128
                                    cur_gs = eg.tile([128, nt, 128], bf16, tag="gs")
                                    nc.gpsimd.dma_gather(
                                        cur_gs[:],
                                        cc_out[cb * BKS:(cb + 1) * BKS,
                                               ct * 128:(ct + 1) * 128],
                                        sidx[:, off // 16:(off + ln) // 16],
                                        num_idxs=ln, num_idxs_reg=ln,
                                        elem_size=128, elem_step=256,
                                    )
                                    cur_gt = eg.tile([128, nt, 128], bf16, tag="gt")
                                    nc.gpsimd.dma_gather(
                                        cur_gt[:],
                                        tgt_tab[:, ct * 128:(ct + 1) * 128],
                                        tidx[:, off // 16:(off + ln) // 16],
                                        num_idxs=ln, num_idxs_reg=ln,
                                        elem_size=128, elem_step=256,
                                    )
                                    call_off = off
                                    ci += 1
                                if edge_mode == "gather_only":
                                    tile_i += 1
                                    continue
                                if edge_mode == "no_gather":
                                    cur_gs, cur_gt = dummy_gs, dummy_gt
                                j = (off - call_off) % GCH // 128
                                p32 = et.tile([128, 128], bf16, tag="p32")
                                nc.vector.tensor_tensor(
                                    p32[:], cur_gs[:, j, :], cur_gt[:, j, :],
                                    op=ALU.add)
                                rbf = et.tile([128, 128], bf16, tag="rbf")
                                nc.scalar.activation(rbf[:], p32[:], AF.Relu)
                                obf = et.tile([128, W], bf16, tag="obf")
                                nc.vector.tensor_scalar(
                                    obf[:], iota[:], keyv[:, tile_i:tile_i + 1],
                                    None, ALU.is_equal)
                                nc.tensor.matmul(st[:], rbf[:], obf[:],
                                                 start=(t == 0),
                                                 stop=(t == Tw - 1))
                                tile_i += 1
                            if edge_mode == "gather_only":
                                continue
                            sl = ST[:, ww * W:(ww + 1) * W]
                            if bb == 0:
                                nc.vector.tensor_copy(sl, st[:])
                            else:
                                nc.vector.tensor_tensor(sl, sl, st[:], op=ALU.add)

                # ---- messages + GRU, per node chunk ----
                with (
                    tc.tile_pool(name=f"gp{layer}", bufs=3) as gp,
                    tc.tile_pool(name=f"mps{layer}", bufs=2, space="PSUM") as mps,
                    tc.tile_pool(name=f"gps{layer}", bufs=1, space="PSUM") as gps,
                ):
                    cs = 0
                    while cs < NSHP:
                        cw = min(512, NSHP - cs)
                        mpsum = mps.tile([128, cw], f32, tag="mpsum")
                        nc.tensor.matmul(mpsum[:], w2sf[:], ST[:, cs:cs + cw],
                                         start=True, stop=False)
                        nc.tensor.matmul(mpsum[:], w2fd[:],
                                         ST[:, KPT + cs:KPT + cs + cw],
                                         start=False, stop=True)
                        bm = gp.tile([128, cw], f32, tag="bm")
                        nc.sync.dma_start(bm[:], biasm_d[:, cs:cs + cw])
                        mT = gp.tile([128, cw], f32, tag="mT")
                        nc.vector.tensor_tensor(mT[:], mpsum[:], bm[:], op=ALU.add)

                        hTc = hT[:, cs:cs + cw]
                        pr = gps.tile([128, cw], f32, tag="pr")
                        nc.tensor.matmul(pr[:], wihT[:, 0:128], mT[:],
                                         start=True, stop=False)
                        nc.tensor.matmul(pr[:], whhT[:, 0:128], hTc,
                                         start=False, stop=True)
                        pz = gps.tile([128, cw], f32, tag="pz")
                        nc.tensor.matmul(pz[:], wihT[:, 128:256], mT[:],
                                         start=True, stop=False)
                        nc.tensor.matmul(pz[:], whhT[:, 128:256], hTc,
                                         start=False, stop=True)
                        pgi = gps.tile([128, cw], f32, tag="pgi")
                        nc.tensor.matmul(pgi[:], wihT[:, 256:384], mT[:],
                                         start=True, stop=True)
                        pgh = gps.tile([128, cw], f32, tag="pgh")
                        nc.tensor.matmul(pgh[:], whhT[:, 256:384], hTc,
                                         start=True, stop=True)

                        r = gp.tile([128, cw], f32, tag="r")
                        nc.scalar.activation(r[:], pr[:], AF.Sigmoid,
                                             bias=gbias[:, 0:1])
                        z = gp.tile([128, cw], f32, tag="z")
                        nc.scalar.activation(z[:], pz[:], AF.Sigmoid,
                                             bias=gbias[:, 1:2])
                        tmp = gp.tile([128, cw], f32, tag="tmp")
                        nc.vector.tensor_tensor(tmp[:], r[:], pgh[:], op=ALU.mult)
                        ad2 = gp.tile([128, cw], f32, tag="ad2")
                        nc.vector.tensor_tensor(ad2[:], pgi[:], tmp[:], op=ALU.add)
                        ng = gp.tile([128, cw], f32, tag="ng")
                        nc.scalar.activation(ng[:], ad2[:], AF.Tanh,
                                             bias=gbias[:, 2:3])
                        d = gp.tile([128, cw], f32, tag="d")
                        nc.vector.tensor_tensor(d[:], hTc, ng[:], op=ALU.subtract)
                        e = gp.tile([128, cw], f32, tag="e")
                        nc.vector.tensor_tensor(e[:], z[:], d[:], op=ALU.mult)
                        nc.vector.tensor_tensor(hTc, ng[:], e[:], op=ALU.add)
                        cs += cw

            # ---- readout ----
            with (
                tc.tile_pool(name="ro", bufs=3) as ro,
                tc.tile_pool(name="rops", bufs=2, space="PSUM") as rops,
                tc.tile_pool(name="lps", bufs=4, space="PSUM") as lps,
            ):
                cs = 0
                while cs < NSHP:
                    cw = min(512, NSHP - cs)
                    rp = rops.tile([128, cw], f32, tag="rp")
                    nc.tensor.matmul(rp[:], wr1[:], hT[:, cs:cs + cw],
                                     start=True, stop=True)
                    r1 = ro.tile([128, cw], f32, tag="r1")
                    nc.scalar.activation(r1[:], rp[:], AF.Relu, bias=br1[:])
                    for j in range(cw // 128):
                        n0 = cs + j * 128
                        nv = min(128, NSH - n0)
                        if nv <= 0:
                            break
                        lg = lps.tile([128, 2], f32, tag="lg")
                        nc.tensor.matmul(lg[:], r1[:, j * 128:(j + 1) * 128],
                                         wr2[:], start=True, stop=True)
                        sm = ro.tile([128, 2], f32, tag="sm")
                        nc.vector.tensor_tensor(sm[:], lg[:], br2[:], op=ALU.add)
                        mx = ro.tile([128, 1], f32, tag="mx")
                        nc.vector.tensor_reduce(mx[:], sm[:],
                                                axis=mybir.AxisListType.X,
                                                op=ALU.max)
                        nmx = ro.tile([128, 1], f32, tag="nmx")
                        nc.vector.tensor_scalar_mul(nmx[:], mx[:], -1.0)
                        ex = ro.tile([128, 2], f32, tag="ex")
                        nc.scalar.activation(ex[:], sm[:], AF.Exp, bias=nmx[:])
                        s = ro.tile([128, 1], f32, tag="s")
                        nc.vector.tensor_reduce(s[:], ex[:],
                                                axis=mybir.AxisListType.X,
                                                op=ALU.add)
                        rs = ro.tile([128, 1], f32, tag="rs")
                        nc.vector.reciprocal(rs[:], s[:])
                        pout = ro.tile([128, 2], f32, tag="pout")
                        nc.vector.tensor_scalar(pout[:], ex[:], rs[:], None,
                                                ALU.mult)
                        nc.sync.dma_start(out_d[n0:n0 + nv, :], pout[:nv, :])
                    cs += cw

    nc.compile()
    return nc


def prepare(**inputs):
    x = np.asarray(inputs["x"], dtype=np.float32)
    edge_index = np.asarray(inputs["edge_index"], dtype=np.int64)
    edge_type = np.asarray(inputs["edge_type"], dtype=np.int64)
    w1_sf = np.asarray(inputs["w1_sf"], np.float32)
    b1_sf = np.asarray(inputs["b1_sf"], np.float32)
    w2_sf = np.asarray(inputs["w2_sf"], np.float32)
    b2_sf = np.asarray(inputs["b2_sf"], np.float32)
    w1_fd = np.asarray(inputs["w1_fd"], np.float32)
    b1_fd = np.asarray(inputs["b1_fd"], np.float32)
    w2_fd = np.asarray(inputs["w2_fd"], np.float32)
    b2_fd = np.asarray(inputs["b2_fd"], np.float32)
    gru_w_ih = np.asarray(inputs["gru_w_ih"], np.float32)
    gru_w_hh = np.asarray(inputs["gru_w_hh"], np.float32)
    gru_b_ih = np.asarray(inputs["gru_b_ih"], np.float32)
    gru_b_hh = np.asarray(inputs["gru_b_hh"], np.float32)
    wr1 = np.asarray(inputs["wr1"], np.float32)
    br1 = np.asarray(inputs["br1"], np.float32)
    wr2 = np.asarray(inputs["wr2"], np.float32)
    br2 = np.asarray(inputs["br2"], np.float32)

    src = edge_index[0].astype(np.int64)
    tgt = edge_index[1].astype(np.int64)
    typ = edge_type.astype(np.int64)

    T, TOT, calls, sidx, tidx, keyv, cnt_t = _preprocess(src, tgt, typ)

    ck = (TOT, tuple(T.reshape(-1).tolist()))
    if ck not in _CACHE:
        _CACHE[ck] = _build(T, TOT, calls)
    nc = _CACHE[ck]

    # ---- weight prep ----
    w1blk = np.concatenate(
        [w1_sf[:H], w1_fd[:H], w1_sf[H:], w1_fd[H:]], axis=1
    ).astype(np.float32)                                   # [128, 512]
    b1t = np.tile(np.concatenate([b1_sf, b1_fd])[None, :], (128, 1)).astype(
        np.float32)                                        # [128, 256]
    wihT = gru_w_ih.T.copy().astype(np.float32)            # [128, 384]
    whhT = gru_w_hh.T.copy().astype(np.float32)
    gb = (gru_b_ih + gru_b_hh).reshape(3, 128).T.copy().astype(np.float32)
    br1c = br1.reshape(128, 1).astype(np.float32)
    br2t = np.tile(br2[None, :], (128, 1)).astype(np.float32)
    import ml_dtypes
    iota = np.tile(np.arange(W, dtype=np.float32), (128, 1)).astype(ml_dtypes.bfloat16)

    common = dict(
        w1blk=w1blk, w2sf=np.ascontiguousarray(w2_sf),
        w2fd=np.ascontiguousarray(w2_fd), wihT=wihT, whhT=whhT, gbias=gb,
        wr1=np.ascontiguousarray(wr1), br1=br1c,
        wr2=np.ascontiguousarray(wr2), br2=br2t, b1t=b1t, iota=iota,
    )

    in_maps = []
    for k in range(NCORE):
        hTk = np.zeros((128, NSHP), np.float32)
        hTk[:, :NSH] = x[k * NSH:(k + 1) * NSH].T
        biasm = np.zeros((128, NSHP), np.float32)
        if b2_sf.any() or b2_fd.any():
            biasm[:, :NSH] = (np.outer(b2_sf, cnt_t[k, 0])
                              + np.outer(b2_fd, cnt_t[k, 1])).astype(np.float32)
        m = dict(common)
        m.update(hT=hTk, biasm=biasm, sidx=sidx[k], tidx=tidx[k], keyv=keyv[k])
        in_maps.append({kk: np.ascontiguousarray(vv) for kk, vv in m.items()})

    return nc, in_maps


def kernel(**inputs):
    nc, in_maps = prepare(**inputs)
    from concourse.bass_utils import run_bass_kernel_spmd
    res = run_bass_kernel_spmd(nc, in_maps, list(range(NCORE)), **_RUN_KWARGS)
    global _LAST_RESULT
    _LAST_RESULT = res
    out = np.concatenate([res.results[k]["out"] for k in range(NCORE)], axis=0)
    return out



# revision 2
# speedup vs baseline: 1.7657x; 1.7657x over previous
"""Trainium2 Bass kernel for a 3-layer edge-typed GNN (message passing + GRU + readout).

Math refactoring (key to the memory-bound regime):
  reference per layer:
    ef = [h[src], h[tgt]]                    # [E, 2H]
    m  = relu(ef @ W1_t) @ W2_t (+biases)    # t = edge type
    messages = segment_sum(m, tgt)
    h = GRU(messages, h)
  Since segment_sum commutes with the second linear layer:
    X_t_src = h @ W1_t[:H] ; X_t_tgt = h @ W1_t[H:] + b1_t    (per-node tables)
    S_t     = segment_sum(relu(X_t_src[src] + X_t_tgt[tgt]), tgt)
    messages = S_sf @ W2_sf + S_fd @ W2_fd (+ counts x b2)
  so ALL per-edge matmuls become per-node matmuls; per edge only
  gather + add + relu + segment-accumulate remain.

Distribution (8 cores):
  - nodes sharded: core k owns nodes [k*6250, (k+1)*6250)
  - edges sharded by TARGET shard -> segment sums are core-local
  - per layer: each core computes X rows for its node shard; the source-side
    halves are AllGathered (one collective per edge type, sf first) into full
    [50000, 128] tables; target-side halves stay resident in SBUF.
  - edge stream order: (bucket0,sf),(bucket1,sf),(bucket0,fd),(bucket1,fd)
    so fd-type edge compute overlaps the fd AllGather.
  - per-edge src rows fetched with dma_gather (int16 idx, 2 row-buckets of
    25000); per-edge tgt rows materialized by PE via host-precomputed
    TRANSPOSED one-hot (OHT) against the SBUF Xt table -> no tgt descriptors.
  - segment-sum via one-hot matmul with HOST-precomputed one-hot tiles (OH),
    streamed from HBM -> no on-device is_equal.
  - GRU + readout data-parallel over node shards, feature-major in SBUF.
"""

import numpy as np

N, E, H, O = 50000, 640000, 128, 2
N_LAYERS = 3
NCORE = 8
NSH = N // NCORE            # 6250 nodes per core
NSHP = 6272                 # padded to 49*128
W = 256                     # segment window width (keys)
KPT = 6400                  # padded keys per type (25 windows of 256)
NWT = KPT // W              # 25 windows per type
NBK = 2                     # source-row buckets (int16 index range)
BKS = N // NBK              # 25000 rows per bucket
GCH = 1024                  # idxs per dma_gather call (SWDGE ring holds 1024 descs)

_CACHE = {}
_RUN_KWARGS = {}
_LAST_RESULT = None


def _preprocess(src, tgt, typ):
    """Build per-core padded edge streams + shared tile structure.

    Segment order: seg = typ*2 + bkt -> (sf,b0),(sf,b1),(fd,b0),(fd,b1).
    """
    core = tgt // NSH
    tgt_l = tgt - core * NSH
    win = tgt_l // W                     # 0..24 within type
    kloc = tgt_l - win * W               # 0..255 key within window
    bkt = src // BKS                     # 0..1

    seg = typ * NBK + bkt                # 0..3
    group = ((core * 4 + seg) * NWT) + win
    perm = np.argsort(group, kind="stable")
    cnt = np.bincount(group, minlength=NCORE * 4 * NWT).reshape(NCORE, 4, NWT)
    T = -(-cnt.max(axis=0) // 128)       # [4, NWT] tiles per (seg, window)
    # first-bucket segments must flush every window's PSUM accumulator
    T[0] = np.maximum(T[0], 1)
    T[2] = np.maximum(T[2], 1)
    TOT = int(T.sum()) * 128

    starts = np.zeros(NCORE * 4 * NWT + 1, np.int64)
    np.cumsum(cnt.reshape(-1), out=starts[1:])
    ss, ks = src[perm], kloc[perm]

    src_s = np.zeros((NCORE, TOT), np.int16)
    key_s = np.full((NCORE, TOT), -1, np.int32)   # -1 = padding slot
    pos = 0
    for sg in range(4):
        bb = sg & 1
        for ww in range(NWT):
            L = int(T[sg, ww]) * 128
            for k in range(NCORE):
                gi = (k * 4 + sg) * NWT + ww
                s0, s1 = int(starts[gi]), int(starts[gi + 1])
                n = s1 - s0
                src_s[k, pos:pos + n] = (ss[s0:s1] - bb * BKS).astype(np.int16)
                key_s[k, pos:pos + n] = ks[s0:s1]
            pos += L
    assert pos == TOT

    # gather-call segmentation: contiguous runs per segment, <= GCH idxs
    calls = []
    pos = 0
    for sg in range(4):
        seg_len = int(T[sg].sum()) * 128
        off = 0
        while off < seg_len:
            ln = min(GCH, seg_len - off)
            calls.append((pos + off, ln, sg))
            off += ln
        pos += seg_len

    def wrap16(a):   # [TOT] -> [128, TOT//16], 16-row stream replicated x8
        return np.tile(a.reshape(-1, 16).T, (8, 1)).copy()

    sidx = np.stack([wrap16(src_s[k]) for k in range(NCORE)])

    # per-core per-type in-degree counts (for b2 bias via counts)
    cnt_t = np.zeros((NCORE, 2, NSH), np.float64)
    np.add.at(cnt_t, (core, typ, tgt_l), 1.0)

    return T, TOT, calls, sidx, key_s, cnt_t


def _make_onehots(key_s):
    """OH [128, NT*256]: per tile [128e, 256k]; OHT: per tile [128k, 2, 128e]."""
    import ml_dtypes
    TOT = key_s.shape[0]
    NT = TOT // 128
    valid = key_s >= 0
    ti = np.arange(TOT) // 128
    ei = np.arange(TOT) % 128
    kk = np.where(valid, key_s, 0)

    oh = np.zeros((NT, 128, 256), np.float32)
    oh[ti[valid], ei[valid], kk[valid]] = 1.0
    oh = np.ascontiguousarray(oh.transpose(1, 0, 2).reshape(128, NT * 256))

    oht = np.zeros((NT, 128, 256), np.float32)
    oht[ti[valid], kk[valid] % 128, (kk[valid] // 128) * 128 + ei[valid]] = 1.0
    oht = np.ascontiguousarray(oht.transpose(1, 0, 2).reshape(128, NT * 256))
    return oh.astype(ml_dtypes.bfloat16), oht.astype(ml_dtypes.bfloat16)


def _build(T, TOT, calls, use_biasm):
    import concourse.bacc as bacc
    import concourse.mybir as mybir
    import concourse.tile as tile

    f32 = mybir.dt.float32
    bf16 = mybir.dt.bfloat16
    i16 = mybir.dt.int16
    AF = mybir.ActivationFunctionType
    ALU = mybir.AluOpType

    nc = bacc.Bacc("TRN2", target_bir_lowering=False, debug=False,
                   num_devices=NCORE)

    # ---- I/O ----
    hT_d = nc.dram_tensor("hT", [128, NSHP], f32, kind="ExternalInput")
    w1blk_d = nc.dram_tensor("w1blk", [128, 512], f32, kind="ExternalInput")
    w2sf_d = nc.dram_tensor("w2sf", [128, 128], bf16, kind="ExternalInput")
    w2fd_d = nc.dram_tensor("w2fd", [128, 128], bf16, kind="ExternalInput")
    wihT_d = nc.dram_tensor("wihT", [128, 384], f32, kind="ExternalInput")
    whhT_d = nc.dram_tensor("whhT", [128, 384], f32, kind="ExternalInput")
    gbias_d = nc.dram_tensor("gbias", [128, 3], f32, kind="ExternalInput")
    wr1_d = nc.dram_tensor("wr1", [128, 128], f32, kind="ExternalInput")
    br1_d = nc.dram_tensor("br1", [128, 1], f32, kind="ExternalInput")
    wr2_d = nc.dram_tensor("wr2", [128, 2], f32, kind="ExternalInput")
    br2_d = nc.dram_tensor("br2", [128, 2], f32, kind="ExternalInput")
    b1t_d = nc.dram_tensor("b1t", [128, 256], f32, kind="ExternalInput")
    sidx_d = nc.dram_tensor("sidx", [128, TOT // 16], i16, kind="ExternalInput")
    oh_d = nc.dram_tensor("oh", [128, 2 * TOT], bf16, kind="ExternalInput")
    oht_d = nc.dram_tensor("oht", [128, 2 * TOT], bf16, kind="ExternalInput")
    if use_biasm:
        biasm_d = nc.dram_tensor("biasm", [128, NSHP], f32,
                                 kind="ExternalInput")
    out_d = nc.dram_tensor("out", [NSH, 2], f32, kind="ExternalOutput")

    cc_in_sf = nc.dram_tensor("cc_in_sf", [NSH, 128], bf16)
    cc_in_fd = nc.dram_tensor("cc_in_fd", [NSH, 128], bf16)
    cc_sf = nc.dram_tensor("cc_sf", [N, 128], bf16, addr_space="Shared")
    cc_fd = nc.dram_tensor("cc_fd", [N, 128], bf16, addr_space="Shared")

    groups = [list(range(NCORE))]
    NCH = NSHP // 128  # 49 node chunks

    with tile.TileContext(nc) as tc:
        with tc.tile_pool(name="persist", bufs=1) as pp:
            hT = pp.tile([128, NSHP], f32)
            w1blk = pp.tile([128, 512], f32)
            w2sf = pp.tile([128, 128], bf16)
            w2fd = pp.tile([128, 128], bf16)
            wihT = pp.tile([128, 384], f32)
            whhT = pp.tile([128, 384], f32)
            gbias = pp.tile([128, 3], f32)
            wr1 = pp.tile([128, 128], f32)
            br1 = pp.tile([128, 1], f32)
            wr2 = pp.tile([128, 2], f32)
            br2 = pp.tile([128, 2], f32)
            b1t = pp.tile([128, 256], f32)
            sidx = pp.tile([128, TOT // 16], i16)
            XtS = pp.tile([128, 100 * 128], bf16)   # [128k, block, 128f]
            ST = pp.tile([128, 2 * KPT], bf16)

            for t_, d_ in [(hT, hT_d), (w1blk, w1blk_d), (w2sf, w2sf_d),
                           (w2fd, w2fd_d), (wihT, wihT_d), (whhT, whhT_d),
                           (gbias, gbias_d), (wr1, wr1_d), (br1, br1_d),
                           (wr2, wr2_d), (br2, br2_d), (b1t, b1t_d),
                           (sidx, sidx_d)]:
                nc.sync.dma_start(t_[:], d_[:])
            # zero XtS once: blocks 49/99 (pad keys) stay zero forever
            nc.vector.memset(XtS[:], 0.0)

            for layer in range(N_LAYERS):
                # ---- X phase: per-node tables X = h @ W1blk ----
                with (
                    tc.tile_pool(name=f"xp{layer}", bufs=3) as xp,
                    tc.tile_pool(name=f"xps{layer}", bufs=2, space="PSUM") as xps,
                ):
                    for c in range(NCH):
                        xpsum = xps.tile([128, 512], f32, tag="xpsum")
                        nc.tensor.matmul(xpsum[:], hT[:, c * 128:(c + 1) * 128],
                                         w1blk[:], start=True, stop=True)
                        xs_sf = xp.tile([128, 128], bf16, tag="xs_sf")
                        nc.scalar.copy(xs_sf[:], xpsum[:, 0:128])
                        xs_fd = xp.tile([128, 128], bf16, tag="xs_fd")
                        nc.scalar.copy(xs_fd[:], xpsum[:, 128:256])
                        nv = min(128, NSH - c * 128)
                        if nv > 0:
                            nc.sync.dma_start(cc_in_sf[c * 128:c * 128 + nv, :],
                                              xs_sf[:nv, :])
                            nc.sync.dma_start(cc_in_fd[c * 128:c * 128 + nv, :],
                                              xs_fd[:nv, :])
                        # Xt blocks: sf -> block c, fd -> block 50+c
                        nc.vector.tensor_tensor(
                            XtS[:, c * 128:(c + 1) * 128],
                            xpsum[:, 256:384], b1t[:, 0:128], op=ALU.add)
                        nc.vector.tensor_tensor(
                            XtS[:, (50 + c) * 128:(51 + c) * 128],
                            xpsum[:, 384:512], b1t[:, 128:256], op=ALU.add)

                nc.gpsimd.collective_compute(
                    "AllGather", mybir.AluOpType.bypass,
                    replica_groups=groups, ins=[cc_in_sf[:]], outs=[cc_sf[:]])
                nc.gpsimd.collective_compute(
                    "AllGather", mybir.AluOpType.bypass,
                    replica_groups=groups, ins=[cc_in_fd[:]], outs=[cc_fd[:]])

                # ---- edge phase ----
                with (
                    tc.tile_pool(name=f"eg{layer}", bufs=4) as eg,
                    tc.tile_pool(name=f"eo{layer}", bufs=3) as eo,
                    tc.tile_pool(name=f"et{layer}", bufs=8) as et,
                    tc.tile_pool(name=f"eps{layer}", bufs=2, space="PSUM") as eps,
                    tc.tile_pool(name=f"gps{layer}", bufs=4, space="PSUM") as gps,
                ):
                    ci = 0
                    call_off = 0
                    cur_gs = None
                    tile_i = 0
                    for sg in range(4):
                        tt, bb = sg >> 1, sg & 1
                        cc_t = cc_sf if tt == 0 else cc_fd
                        for ww in range(NWT):
                            Tw = int(T[sg, ww])
                            if Tw == 0:
                                continue
                            # one batched DMA per (window,seg) run for OH/OHT
                            ohr = eo.tile([128, Tw * 256], bf16, tag="ohr")
                            nc.sync.dma_start(
                                ohr[:], oh_d[:, tile_i * 256:(tile_i + Tw) * 256])
                            ohtr = eo.tile([128, Tw * 256], bf16, tag="ohtr")
                            nc.sync.dma_start(
                                ohtr[:],
                                oht_d[:, tile_i * 256:(tile_i + Tw) * 256])
                            blo = (tt * 50 + 2 * ww) * 128
                            bhi = blo + 128
                            st = eps.tile([128, W], f32, tag="st")
                            for t in range(Tw):
                                off = tile_i * 128
                                if ci < len(calls) and calls[ci][0] == off:
                                    _, ln, _ = calls[ci]
                                    nt = ln // 128
                                    cur_gs = eg.tile([128, nt, 128], bf16,
                                                     tag="gs")
                                    nc.gpsimd.dma_gather(
                                        cur_gs[:],
                                        cc_t[bb * BKS:(bb + 1) * BKS, :],
                                        sidx[:, off // 16:(off + ln) // 16],
                                        num_idxs=ln, num_idxs_reg=ln,
                                        elem_size=128, elem_step=128,
                                    )
                                    call_off = off
                                    ci += 1
                                j = (off - call_off) % GCH // 128
                                gt = gps.tile([128, 128], f32, tag="gt")
                                o0 = t * 256
                                nc.tensor.matmul(gt[:], ohtr[:, o0:o0 + 128],
                                                 XtS[:, blo:blo + 128],
                                                 start=True, stop=False)
                                nc.tensor.matmul(gt[:], ohtr[:, o0 + 128:o0 + 256],
                                                 XtS[:, bhi:bhi + 128],
                                                 start=False, stop=True)
                                ad = et.tile([128, 128], bf16, tag="ad")
                                nc.vector.tensor_tensor(ad[:], gt[:],
                                                        cur_gs[:, j, :],
                                                        op=ALU.add)
                                rbf = et.tile([128, 128], bf16, tag="rbf")
                                nc.scalar.activation(rbf[:], ad[:], AF.Relu)
                                nc.tensor.matmul(st[:], rbf[:],
                                                 ohr[:, o0:o0 + 256],
                                                 start=(t == 0),
                                                 stop=(t == Tw - 1))
                                tile_i += 1
                            sl = ST[:, (tt * KPT + ww * W):(tt * KPT + (ww + 1) * W)]
                            if bb == 0:
                                nc.vector.tensor_copy(sl, st[:])
                            else:
                                nc.vector.tensor_tensor(sl, sl, st[:], op=ALU.add)

                # ---- messages + GRU, per node chunk ----
                with (
                    tc.tile_pool(name=f"gp{layer}", bufs=3) as gp,
                    tc.tile_pool(name=f"mps{layer}", bufs=2, space="PSUM") as mps,
                    tc.tile_pool(name=f"gps2{layer}", bufs=1, space="PSUM") as gps2,
                ):
                    cs = 0
                    while cs < NSHP:
                        cw = min(512, NSHP - cs)
                        mpsum = mps.tile([128, cw], f32, tag="mpsum")
                        nc.tensor.matmul(mpsum[:], w2sf[:], ST[:, cs:cs + cw],
                                         start=True, stop=False)
                        nc.tensor.matmul(mpsum[:], w2fd[:],
                                         ST[:, KPT + cs:KPT + cs + cw],
                                         start=False, stop=True)
                        mT = gp.tile([128, cw], f32, tag="mT")
                        if use_biasm:
                            bm = gp.tile([128, cw], f32, tag="bm")
                            nc.sync.dma_start(bm[:], biasm_d[:, cs:cs + cw])
                            nc.vector.tensor_tensor(mT[:], mpsum[:], bm[:],
                                                    op=ALU.add)
                        else:
                            nc.vector.tensor_copy(mT[:], mpsum[:])

                        hTc = hT[:, cs:cs + cw]
                        pr = gps2.tile([128, cw], f32, tag="pr")
                        nc.tensor.matmul(pr[:], wihT[:, 0:128], mT[:],
                                         start=True, stop=False)
                        nc.tensor.matmul(pr[:], whhT[:, 0:128], hTc,
                                         start=False, stop=True)
                        pz = gps2.tile([128, cw], f32, tag="pz")
                        nc.tensor.matmul(pz[:], wihT[:, 128:256], mT[:],
                                         start=True, stop=False)
                        nc.tensor.matmul(pz[:], whhT[:, 128:256], hTc,
                                         start=False, stop=True)
                        pgi = gps2.tile([128, cw], f32, tag="pgi")
                        nc.tensor.matmul(pgi[:], wihT[:, 256:384], mT[:],
                                         start=True, stop=True)
                        pgh = gps2.tile([128, cw], f32, tag="pgh")
                        nc.tensor.matmul(pgh[:], whhT[:, 256:384], hTc,
                                         start=True, stop=True)

                        r = gp.tile([128, cw], f32, tag="r")
                        nc.scalar.activation(r[:], pr[:], AF.Sigmoid,
                                             bias=gbias[:, 0:1])
                        z = gp.tile([128, cw], f32, tag="z")
                        nc.scalar.activation(z[:], pz[:], AF.Sigmoid,
                                             bias=gbias[:, 1:2])
                        tmp = gp.tile([128, cw], f32, tag="tmp")
                        nc.vector.tensor_tensor(tmp[:], r[:], pgh[:], op=ALU.mult)
                        ad2 = gp.tile([128, cw], f32, tag="ad2")
                        nc.vector.tensor_tensor(ad2[:], pgi[:], tmp[:], op=ALU.add)
                        ng = gp.tile([128, cw], f32, tag="ng")
                        nc.scalar.activation(ng[:], ad2[:], AF.Tanh,
                                             bias=gbias[:, 2:3])
                        d = gp.tile([128, cw], f32, tag="d")
                        nc.vector.tensor_tensor(d[:], hTc, ng[:], op=ALU.subtract)
                        e = gp.tile([128, cw], f32, tag="e")
                        nc.vector.tensor_tensor(e[:], z[:], d[:], op=ALU.mult)
                        nc.vector.tensor_tensor(hTc, ng[:], e[:], op=ALU.add)
                        cs += cw

            # ---- readout ----
            with (
                tc.tile_pool(name="ro", bufs=3) as ro,
                tc.tile_pool(name="rops", bufs=2, space="PSUM") as rops,
                tc.tile_pool(name="lps", bufs=4, space="PSUM") as lps,
            ):
                cs = 0
                while cs < NSHP:
                    cw = min(512, NSHP - cs)
                    rp = rops.tile([128, cw], f32, tag="rp")
                    nc.tensor.matmul(rp[:], wr1[:], hT[:, cs:cs + cw],
                                     start=True, stop=True)
                    r1 = ro.tile([128, cw], f32, tag="r1")
                    nc.scalar.activation(r1[:], rp[:], AF.Relu, bias=br1[:])
                    for j in range(cw // 128):
                        n0 = cs + j * 128
                        nv = min(128, NSH - n0)
                        if nv <= 0:
                            break
                        lg = lps.tile([128, 2], f32, tag="lg")
                        nc.tensor.matmul(lg[:], r1[:, j * 128:(j + 1) * 128],
                                         wr2[:], start=True, stop=True)
                        sm = ro.tile([128, 2], f32, tag="sm")
                        nc.vector.tensor_tensor(sm[:], lg[:], br2[:], op=ALU.add)
                        mx = ro.tile([128, 1], f32, tag="mx")
                        nc.vector.tensor_reduce(mx[:], sm[:],
                                                axis=mybir.AxisListType.X,
                                                op=ALU.max)
                        nmx = ro.tile([128, 1], f32, tag="nmx")
                        nc.vector.tensor_scalar_mul(nmx[:], mx[:], -1.0)
                        ex = ro.tile([128, 2], f32, tag="ex")
                        nc.scalar.activation(ex[:], sm[:], AF.Exp, bias=nmx[:])
                        s = ro.tile([128, 1], f32, tag="s")
                        nc.vector.tensor_reduce(s[:], ex[:],
                                                axis=mybir.AxisListType.X,
                                                op=ALU.add)
                        rs = ro.tile([128, 1], f32, tag="rs")
                        nc.vector.reciprocal(rs[:], s[:])
                        pout = ro.tile([128, 2], f32, tag="pout")
                        nc.vector.tensor_scalar(pout[:], ex[:], rs[:], None,
                                                ALU.mult)
                        nc.sync.dma_start(out_d[n0:n0 + nv, :], pout[:nv, :])
                    cs += cw

    nc.compile()
    return nc


def prepare(**inputs):
    import ml_dtypes
    x = np.asarray(inputs["x"], dtype=np.float32)
    edge_index = np.asarray(inputs["edge_index"], dtype=np.int64)
    edge_type = np.asarray(inputs["edge_type"], dtype=np.int64)
    w1_sf = np.asarray(inputs["w1_sf"], np.float32)
    b1_sf = np.asarray(inputs["b1_sf"], np.float32)
    w2_sf = np.asarray(inputs["w2_sf"], np.float32)
    b2_sf = np.asarray(inputs["b2_sf"], np.float32)
    w1_fd = np.asarray(inputs["w1_fd"], np.float32)
    b1_fd = np.asarray(inputs["b1_fd"], np.float32)
    w2_fd = np.asarray(inputs["w2_fd"], np.float32)
    b2_fd = np.asarray(inputs["b2_fd"], np.float32)
    gru_w_ih = np.asarray(inputs["gru_w_ih"], np.float32)
    gru_w_hh = np.asarray(inputs["gru_w_hh"], np.float32)
    gru_b_ih = np.asarray(inputs["gru_b_ih"], np.float32)
    gru_b_hh = np.asarray(inputs["gru_b_hh"], np.float32)
    wr1 = np.asarray(inputs["wr1"], np.float32)
    br1 = np.asarray(inputs["br1"], np.float32)
    wr2 = np.asarray(inputs["wr2"], np.float32)
    br2 = np.asarray(inputs["br2"], np.float32)

    src = edge_index[0].astype(np.int64)
    tgt = edge_index[1].astype(np.int64)
    typ = edge_type.astype(np.int64)

    T, TOT, calls, sidx, key_s, cnt_t = _preprocess(src, tgt, typ)
    use_biasm = bool(b2_sf.any() or b2_fd.any())

    ck = (TOT, use_biasm, tuple(T.reshape(-1).tolist()))
    if ck not in _CACHE:
        _CACHE[ck] = _build(T, TOT, calls, use_biasm)
    nc = _CACHE[ck]

    # ---- weight prep ----
    w1blk = np.concatenate(
        [w1_sf[:H], w1_fd[:H], w1_sf[H:], w1_fd[H:]], axis=1
    ).astype(np.float32)                                   # [128, 512]
    b1t = np.tile(np.concatenate([b1_sf, b1_fd])[None, :], (128, 1)).astype(
        np.float32)                                        # [128, 256]
    wihT = gru_w_ih.T.copy().astype(np.float32)            # [128, 384]
    whhT = gru_w_hh.T.copy().astype(np.float32)
    gb = (gru_b_ih + gru_b_hh).reshape(3, 128).T.copy().astype(np.float32)
    br1c = br1.reshape(128, 1).astype(np.float32)
    br2t = np.tile(br2[None, :], (128, 1)).astype(np.float32)

    common = dict(
        w1blk=w1blk,
        w2sf=np.ascontiguousarray(w2_sf).astype(ml_dtypes.bfloat16),
        w2fd=np.ascontiguousarray(w2_fd).astype(ml_dtypes.bfloat16),
        wihT=wihT, whhT=whhT, gbias=gb,
        wr1=np.ascontiguousarray(wr1), br1=br1c,
        wr2=np.ascontiguousarray(wr2), br2=br2t, b1t=b1t,
    )

    in_maps = []
    for k in range(NCORE):
        hTk = np.zeros((128, NSHP), np.float32)
        hTk[:, :NSH] = x[k * NSH:(k + 1) * NSH].T
        oh_k, oht_k = _make_onehots(key_s[k])
        m = dict(common)
        m.update(hT=hTk, sidx=sidx[k], oh=oh_k, oht=oht_k)
        if use_biasm:
            biasm = np.zeros((128, NSHP), np.float32)
            biasm[:, :NSH] = (np.outer(b2_sf, cnt_t[k, 0])
                              + np.outer(b2_fd, cnt_t[k, 1])).astype(np.float32)
            m.update(biasm=biasm)
        in_maps.append({kk: np.ascontiguousarray(vv) for kk, vv in m.items()})

    return nc, in_maps


def kernel(**inputs):
    nc, in_maps = prepare(**inputs)
    from concourse.bass_utils import run_bass_kernel_spmd
    res = run_bass_kernel_spmd(nc, in_maps, list(range(NCORE)), **_RUN_KWARGS)
    global _LAST_RESULT
    _LAST_RESULT = res
    out = np.concatenate([res.results[k]["out"] for k in range(NCORE)], axis=0)
    return out
